# revision 10
# baseline (speedup 1.0000x reference)
"""BEVFusion-style LSS bev_pool on 8 Trainium2 NeuronCores.

Algorithm
---------
The reference projects a (B=1, N=6, D=118, FH=32, FW=88) frustum of points
into a 360x360 BEV grid (NZ=1) and scatter-sums each point's C=80 feature
vector into its voxel.  The geometry (which voxel each point hits) depends
only on the tiny 4x4 matrices, so it is computed on host; the memory-heavy
part - streaming the 638MB feature tensor and scatter-summing - runs on the
8 NeuronCores.

Device kernel: points are sorted by (grid_row cx, cy-block-of-128) into
buckets on host, split into chunks of <= SMAX 128-point tiles, and
load-balanced across cores.  Every core runs the SAME instruction stream
(SPMD): for each 128-point tile, a one-hot matrix over the 128-wide cy block
is built on the Vector engine (iota vs cy compare), and the TensorEngine
matmul  onehot^T(128pt x 128vy) @ x(128pt x 80ch)  scatter-accumulates the
tile into a PSUM bank.  Chunks accumulate over their tiles in PSUM, are
copied to SBUF and DMA'd out as (slot, 128, 80) partial sums, which the host
adds into the final grid.  Features are streamed as bf16 (PSUM accumulation
stays fp32).
"""

import os
import numpy as np
import ml_dtypes

import jax
import jax.numpy as jnp

import concourse.bacc as bacc
import concourse.mybir as mybir
import concourse.tile as tile
from concourse.bass_utils import run_bass_kernel_spmd

# problem constants (hardcoded per contract)
B, N, D, FH, FW, C = 1, 6, 118, 32, 88, 80
IH, IW = 256, 704
XB = (-54.0, 54.0, 0.3)
YB = (-54.0, 54.0, 0.3)
ZB = (-10.0, 10.0, 20.0)
DB = (1.0, 60.0, 0.5)
NX, NY, NZ = 360, 360, 1

P = 128          # points per tile (matmul contraction)
W = 64           # one-hot width = cy block size; NBLK blocks cover NY
NBLK = (NY + W - 1) // W            # 3
SMAX = 24        # max tiles per chunk (one PSUM accumulation group)
BT = 64          # tiles per DMA batch / one-hot batch
NCORES = 8

_cache = {}


def _frustum():
    ds = jnp.arange(DB[0], DB[1], DB[2], dtype=jnp.float32)
    xs = jnp.linspace(0.0, IW - 1.0, FW, dtype=jnp.float32)
    ys = jnp.linspace(0.0, IH - 1.0, FH, dtype=jnp.float32)
    ds = jnp.broadcast_to(ds[:, None, None], (D, FH, FW))
    xs = jnp.broadcast_to(xs[None, None, :], (D, FH, FW))
    ys = jnp.broadcast_to(ys[None, :, None], (D, FH, FW))
    return jnp.stack([xs, ys, ds], axis=-1)


def _geometry(camera2lidar, camera_intrinsics, img_aug_matrix, lidar_aug_matrix):
    """Mirror of the reference get_geometry + voxel-coord computation, run on
    CPU jax so the fp32 arithmetic matches the reference bit-for-bit."""
    cpu = jax.devices("cpu")[0]
    with jax.default_device(cpu):
        camera2lidar = jnp.asarray(np.asarray(camera2lidar))
        camera_intrinsics = jnp.asarray(np.asarray(camera_intrinsics))
        img_aug_matrix = jnp.asarray(np.asarray(img_aug_matrix))
        lidar_aug_matrix = jnp.asarray(np.asarray(lidar_aug_matrix))

        rots = camera2lidar[..., :3, :3]
        trans = camera2lidar[..., :3, 3]
        intrins = camera_intrinsics[..., :3, :3]
        post_rots = img_aug_matrix[..., :3, :3]
        post_trans = img_aug_matrix[..., :3, 3]
        extra_rots = lidar_aug_matrix[..., :3, :3]
        extra_trans = lidar_aug_matrix[..., :3, 3]

        pts = _frustum()[None, None] - post_trans[:, :, None, None, None, :]
        pts = jnp.einsum('bnji,bndhwj->bndhwi', post_rots, pts)
        pts = jnp.concatenate([pts[..., :2] * pts[..., 2:3], pts[..., 2:3]], axis=-1)
        combine = jnp.swapaxes(
            jnp.linalg.solve(jnp.swapaxes(intrins, -1, -2), jnp.swapaxes(rots, -1, -2)),
            -1, -2)
        pts = jnp.einsum('bnij,bndhwj->bndhwi', combine, pts)
        pts = pts + trans[:, :, None, None, None, :]
        pts = jnp.einsum('bij,bndhwj->bndhwi', extra_rots, pts)
        pts = pts + extra_trans[:, None, None, None, None, :]

        dx = jnp.array([XB[2], YB[2], ZB[2]], dtype=jnp.float32)
        bx = jnp.array([XB[0] + XB[2] / 2.0, YB[0] + YB[2] / 2.0,
                        ZB[0] + ZB[2] / 2.0], dtype=jnp.float32)
        coords = ((pts - (bx - dx / 2.0)) / dx).astype(jnp.int32)
        coords = np.asarray(jax.device_get(coords))

    cx = coords[..., 0].ravel()
    cy = coords[..., 1].ravel()
    cz = coords[..., 2].ravel()
    kept = (cx >= 0) & (cx < NX) & (cy >= 0) & (cy < NY) & (cz >= 0) & (cz < NZ)
    return kept, cx, cy


def _plan(kept, cx, cy):
    """Sort kept points into (cx, cy-block) buckets, chunk, and balance across
    cores.  Returns per-core packed point-index grids, the shared chunk-length
    schedule, and per-(core, slot) bucket ids."""
    kidx = np.nonzero(kept)[0].astype(np.int64)
    kcx = cx[kidx].astype(np.int64)
    kcy = cy[kidx].astype(np.int64)
    bucket = kcx * NBLK + (kcy // W)
    order = np.argsort(bucket, kind="stable")
    sidx = kidx[order]                    # point indices sorted by bucket
    sbucket = bucket[order]
    cnt = np.bincount(sbucket, minlength=NX * NBLK)
    starts = np.concatenate([[0], np.cumsum(cnt)])

    # chunks: (bucket, ntiles, point_start, npts)
    chunks = []
    for b in np.nonzero(cnt)[0]:
        off = starts[b]
        rem = int(cnt[b])
        while rem > 0:
            npts = min(rem, SMAX * P)
            chunks.append((int(b), (npts + P - 1) // P, int(off), npts))
            off += npts
            rem -= npts
    chunks.sort(key=lambda c: -c[1])

    loads = [0] * NCORES
    percore = [[] for _ in range(NCORES)]
    for ch in chunks:
        k = loads.index(min(loads))
        percore[k].append(ch)
        loads[k] += ch[1]

    nslot = max(len(p) for p in percore)
    sched = [max(p[i][1] if i < len(p) else 0 for p in percore)
             for i in range(nslot)]
    ntile = sum(sched)
    nbatch = (ntile + BT - 1) // BT
    ntile_pad = nbatch * BT

    # per-core point-index grid (ntile_pad, P): -1 = padding
    idx_grids = []
    slot_buckets = []
    for k in range(NCORES):
        grid = np.full((ntile_pad, P), -1, dtype=np.int64)
        sb = np.full(nslot, -1, dtype=np.int64)
        t0 = 0
        for i, slen in enumerate(sched):
            if i < len(percore[k]):
                b, nt, off, npts = percore[k][i]
                sb[i] = b
                flat = grid[t0:t0 + slen].reshape(-1)
                flat[:npts] = sidx[off:off + npts]
            t0 += slen
        idx_grids.append(grid)
        slot_buckets.append(sb)
    return idx_grids, slot_buckets, sched, nbatch, nslot, ntile_pad


def _build(sched, nbatch, nslot, reps=1, mode="full"):
    """Build the SPMD Bacc program for the shared schedule.  reps>1 wraps the
    body in a device-side loop; mode strips stages (both timing-only)."""
    ntile_pad = nbatch * BT
    nc = bacc.Bacc("TRN2", target_bir_lowering=False, num_devices=NCORES)
    xpk = nc.dram_tensor("xpk", [nbatch, P, BT * C], mybir.dt.bfloat16,
                         kind="ExternalInput")
    cypk = nc.dram_tensor("cypk", [P, ntile_pad + W], mybir.dt.bfloat16,
                          kind="ExternalInput")
    out = nc.dram_tensor("out", [nslot, W, C], mybir.dt.float32,
                         kind="ExternalOutput")

    with tile.TileContext(nc) as tc:
        from contextlib import ExitStack, nullcontext
        with (
            tc.tile_pool(name="const", bufs=1) as const_pool,
            tc.tile_pool(name="x", bufs=3) as x_pool,
            tc.tile_pool(name="oh", bufs=3) as oh_pool,
            tc.tile_pool(name="cp", bufs=4) as cp_pool,
            tc.tile_pool(name="ps", bufs=8, space="PSUM") as ps_pool,
        ):
            const_t = const_pool.tile([P, ntile_pad + W], mybir.dt.bfloat16)
            nc.sync.dma_start(out=const_t[:], in_=cypk[:, :])
            cy_t = const_t[:, :ntile_pad]
            iota_t = const_t[:, ntile_pad:]

            loop_cm = tc.For_i(0, reps, 1) if reps > 1 else nullcontext()
            with loop_cm:
                x_tiles = [None] * ntile_pad
                oh_tiles = [None] * ntile_pad
                staged = [False] * nbatch

                def stage_batch(bi):
                    x_b = x_pool.tile([P, BT * C], mybir.dt.bfloat16)
                    nc.sync.dma_start(out=x_b[:], in_=xpk[bi, :, :])
                    oh_b = oh_pool.tile([P, BT * W], mybir.dt.bfloat16)
                    t0 = bi * BT
                    eng = nc.vector
                    if mode != "dma":
                        eng.tensor_tensor(
                            out=oh_b[:].rearrange("p (t w) -> p t w", w=W),
                            in0=cy_t[:, t0:t0 + BT][:, :, None].to_broadcast([P, BT, W]),
                            in1=iota_t[:, None, :].to_broadcast([P, BT, W]),
                            op=mybir.AluOpType.is_equal,
                        )
                    for j in range(BT):
                        x_tiles[t0 + j] = x_b[:, j * C:(j + 1) * C]
                        oh_tiles[t0 + j] = oh_b[:, j * W:(j + 1) * W]
                    staged[bi] = True

                t = 0
                for si, slen in enumerate(sched):
                    psum_t = ps_pool.tile([W, C], mybir.dt.float32, space="PSUM")
                    for j in range(slen):
                        bi = t // BT
                        if not staged[bi]:
                            stage_batch(bi)
                        if mode not in ("dma", "dma_tt"):
                            nc.tensor.matmul(
                                out=psum_t[:], lhsT=oh_tiles[t], rhs=x_tiles[t],
                                start=(j == 0), stop=(j == slen - 1),
                            )
                        t += 1
                    if mode == "full" or (mode == "dma_tt_mm" and si == len(sched) - 1):
                        cp_t = cp_pool.tile([W, C], mybir.dt.float32)
                        nc.scalar.copy(out=cp_t[:], in_=psum_t[:])
                        nc.sync.dma_start(out=out[si, :, :], in_=cp_t[:])
                    elif mode in ("dma", "dma_tt") and si == len(sched) - 1:
                        cp_t = cp_pool.tile([W, C], mybir.dt.float32)
                        nc.scalar.copy(out=cp_t[:], in_=x_tiles[0])
                        nc.sync.dma_start(out=out[si, :, :], in_=cp_t[:])
    nc.compile()
    return nc


def _prepare(x, camera2lidar, camera_intrinsics, img_aug_matrix, lidar_aug_matrix):
    x = np.asarray(x)
    kept, cx, cy = _geometry(camera2lidar, camera_intrinsics,
                             img_aug_matrix, lidar_aug_matrix)
    idx_grids, slot_buckets, sched, nbatch, nslot, ntile_pad = _plan(kept, cx, cy)

    key = (tuple(sched), nbatch, nslot)
    if key not in _cache:
        _cache.clear()
        _cache[key] = _build(sched, nbatch, nslot)
    nc = _cache[key]

    xb = x.reshape(-1, C).astype(ml_dtypes.bfloat16)
    cyb = np.where(kept, cy & (W - 1), 0).astype(ml_dtypes.bfloat16)
    iota_np = np.broadcast_to(np.arange(W, dtype=np.float32), (P, W)) \
        .astype(ml_dtypes.bfloat16)

    in_maps = []
    for k in range(NCORES):
        grid = idx_grids[k]                       # (ntile_pad, P)
        safe = np.maximum(grid, 0)
        xg = xb[safe.reshape(-1)].reshape(ntile_pad, P, C)
        xg[grid.reshape(ntile_pad, P) < 0] = 0
        xpk = np.ascontiguousarray(
            xg.reshape(nbatch, BT, P, C).transpose(0, 2, 1, 3)
              .reshape(nbatch, P, BT * C))
        cyg = cyb[safe]                           # (ntile_pad, P) bf16
        cyg[grid < 0] = 0
        cypk = np.concatenate([cyg.T, iota_np], axis=1)  # (P, ntile_pad + W)
        in_maps.append({
            "xpk": xpk,
            "cypk": np.ascontiguousarray(cypk),
        })
    return nc, in_maps, slot_buckets, nslot


def _assemble(results, slot_buckets, nslot):
    outg = np.zeros((NY, NX, C), dtype=np.float32)   # (y, x, c) for fast adds
    for k in range(NCORES):
        part = results[k]["out"]                     # (nslot, W, C)
        for si in range(nslot):
            b = int(slot_buckets[k][si])
            if b < 0:
                continue
            gx, blk = divmod(b, NBLK)
            y0 = blk * W
            wv = min(W, NY - y0)
            outg[y0:y0 + wv, gx, :] += part[si, :wv, :]
    out = outg.transpose(2, 1, 0)                    # (C, NX, NY)
    return np.ascontiguousarray(out[None]).reshape(B, C * NZ, NX, NY)


def kernel(x, camera2lidar, camera_intrinsics, img_aug_matrix, lidar_aug_matrix):
    nc, in_maps, slot_buckets, nslot = _prepare(
        x, camera2lidar, camera_intrinsics, img_aug_matrix, lidar_aug_matrix)
    res = run_bass_kernel_spmd(nc, in_maps, core_ids=list(range(NCORES)))
    return _assemble(res.results, slot_buckets, nslot)


def _make_runner(nc, in_maps):
    """Build a reusable jitted PJRT runner for a compiled Bacc program.
    Returns run(), where run() executes once and returns wall seconds."""
    import time
    from jax.sharding import Mesh, PartitionSpec, NamedSharding
    from jax.experimental.shard_map import shard_map
    from concourse import bass2jax, mybir as _mb

    bass2jax.install_neuronx_cc_hook()

    partition_name = nc.partition_id_tensor.name if nc.partition_id_tensor else None
    in_names, out_names, out_avals, zero_outs = [], [], [], []
    for alloc in nc.m.functions[0].allocations:
        if not isinstance(alloc, _mb.MemoryLocationSet):
            continue
        name = alloc.memorylocations[0].name
        if alloc.kind == "ExternalInput":
            if name != partition_name:
                in_names.append(name)
        elif alloc.kind == "ExternalOutput":
            shape = tuple(alloc.tensor_shape)
            dtype = _mb.dt.np(alloc.dtype)
            out_names.append(name)
            out_avals.append(jax.core.ShapedArray(shape, dtype))
            zero_outs.append(np.zeros(shape, dtype))
    n_params = len(in_names)
    n_outs = len(out_avals)
    all_in_names = list(in_names) + list(out_names)
    if partition_name is not None:
        all_in_names.append(partition_name)

    def _body(*args):
        operands = list(args)
        if partition_name is not None:
            operands.append(bass2jax.partition_id_tensor())
        outs = bass2jax._bass_exec_p.bind(
            *operands,
            out_avals=tuple(out_avals),
            in_names=tuple(all_in_names),
            out_names=tuple(out_names),
            lowering_input_output_aliases=(),
            sim_require_finite=True,
            sim_require_nnan=True,
            nc=nc,
        )
        return tuple(outs)

    devices = jax.devices()[:NCORES]
    mesh = Mesh(np.asarray(devices), ("core",))
    in_specs = (PartitionSpec("core"),) * (n_params + n_outs)
    out_specs = (PartitionSpec("core"),) * n_outs
    donate = tuple(range(n_params, n_params + n_outs))
    sharded = jax.jit(
        shard_map(_body, mesh=mesh, in_specs=in_specs, out_specs=out_specs,
                  check_rep=False),
        donate_argnums=donate, keep_unused=True)

    shard = NamedSharding(mesh, PartitionSpec("core"))
    concat_in = [
        jax.device_put(
            np.concatenate([np.asarray(in_maps[c][nm]) for c in range(NCORES)], axis=0),
            shard)
        for nm in in_names
    ]
    concat_zero_np = [
        np.zeros((NCORES * z.shape[0], *z.shape[1:]), z.dtype) for z in zero_outs
    ]

    def run():
        zeros_dev = [jax.device_put(z, shard) for z in concat_zero_np]
        jax.block_until_ready(zeros_dev)
        t0 = time.perf_counter()
        outs = sharded(*concat_in, *zeros_dev)
        jax.block_until_ready(outs)
        return time.perf_counter() - t0

    return run


def bench(inputs, iters=20, reps=64):
    """Estimate per-execution HW time by comparing a NEFF that runs the body
    `reps` times in a device-side loop against the single-shot NEFF (both
    measured as min wall-clock over iters, subtracting the shared ~76ms axon
    dispatch overhead via the slope)."""
    nc1, in_maps, _, _ = _prepare(**inputs)
    kept_key = next(iter(_cache))
    sched, nbatch, nslot = list(kept_key[0]), kept_key[1], kept_key[2]
    ncR = _build(sched, nbatch, nslot, reps=reps)

    run1 = _make_runner(nc1, in_maps)
    runR = _make_runner(ncR, in_maps)

    run1(); runR()                       # warmup/compile
    # run each program's iterations consecutively: alternating executables
    # forces a NEFF reload (~40ms) that poisons the measurement
    t1s = [run1() for _ in range(iters)]
    tRs = [runR() for _ in range(iters)]
    t1s.sort(); tRs.sort()
    t1, tR = t1s[0], tRs[0]
    per = (tR - t1) / (reps - 1)
    print(f"bench: single {t1*1e3:.2f} ms, x{reps} {tR*1e3:.2f} ms "
          f"-> per-exec {per*1e6:.1f} us")
    return per * 1e9


# revision 19
# speedup vs baseline: 2.4859x; 2.4859x over previous
"""BEVFusion-style LSS bev_pool on 8 Trainium2 NeuronCores.

Algorithm
---------
The reference projects a (B=1, N=6, D=118, FH=32, FW=88) frustum of points
into a 360x360 BEV grid (NZ=1) and scatter-sums each point's C=80 feature
vector into its voxel.  The geometry (which voxel each point hits) depends
only on the tiny 4x4 matrices, so it is computed on host; the memory-heavy
part - streaming the 638MB feature tensor and scatter-summing - runs on the
8 NeuronCores.

Device kernel: points are sorted by (grid_row cx, cy-block-of-128) into
buckets on host, split into chunks of <= SMAX 128-point tiles, and
load-balanced across cores.  Every core runs the SAME instruction stream
(SPMD): for each 128-point tile, a one-hot matrix over the 128-wide cy block
is built on the Vector engine (iota vs cy compare), and the TensorEngine
matmul  onehot^T(128pt x 128vy) @ x(128pt x 80ch)  scatter-accumulates the
tile into a PSUM bank.  Chunks accumulate over their tiles in PSUM, are
copied to SBUF and DMA'd out as (slot, 128, 80) partial sums, which the host
adds into the final grid.  Features are streamed as bf16 (PSUM accumulation
stays fp32).
"""

import os
import numpy as np
import ml_dtypes

import jax
import jax.numpy as jnp

import concourse.bacc as bacc
import concourse.mybir as mybir
import concourse.tile as tile
from concourse.bass_utils import run_bass_kernel_spmd

# problem constants (hardcoded per contract)
B, N, D, FH, FW, C = 1, 6, 118, 32, 88, 80
IH, IW = 256, 704
XB = (-54.0, 54.0, 0.3)
YB = (-54.0, 54.0, 0.3)
ZB = (-10.0, 10.0, 20.0)
DB = (1.0, 60.0, 0.5)
NX, NY, NZ = 360, 360, 1

P = 128          # points per tile (matmul contraction)
W = 64           # one-hot width = cy block size; NBLK blocks cover NY
NBLK = (NY + W - 1) // W            # 3
SMAX = 24        # max tiles per chunk (one PSUM accumulation group)
BT = 128         # tiles per DMA batch / one-hot batch
PSG = 6          # output slots packed per PSUM bank (PSG*C*4 <= 2KB)
NCORES = 8

_cache = {}


def _frustum():
    ds = jnp.arange(DB[0], DB[1], DB[2], dtype=jnp.float32)
    xs = jnp.linspace(0.0, IW - 1.0, FW, dtype=jnp.float32)
    ys = jnp.linspace(0.0, IH - 1.0, FH, dtype=jnp.float32)
    ds = jnp.broadcast_to(ds[:, None, None], (D, FH, FW))
    xs = jnp.broadcast_to(xs[None, None, :], (D, FH, FW))
    ys = jnp.broadcast_to(ys[None, :, None], (D, FH, FW))
    return jnp.stack([xs, ys, ds], axis=-1)


def _geometry(camera2lidar, camera_intrinsics, img_aug_matrix, lidar_aug_matrix):
    """Mirror of the reference get_geometry + voxel-coord computation, run on
    CPU jax so the fp32 arithmetic matches the reference bit-for-bit."""
    cpu = jax.devices("cpu")[0]
    with jax.default_device(cpu):
        camera2lidar = jnp.asarray(np.asarray(camera2lidar))
        camera_intrinsics = jnp.asarray(np.asarray(camera_intrinsics))
        img_aug_matrix = jnp.asarray(np.asarray(img_aug_matrix))
        lidar_aug_matrix = jnp.asarray(np.asarray(lidar_aug_matrix))

        rots = camera2lidar[..., :3, :3]
        trans = camera2lidar[..., :3, 3]
        intrins = camera_intrinsics[..., :3, :3]
        post_rots = img_aug_matrix[..., :3, :3]
        post_trans = img_aug_matrix[..., :3, 3]
        extra_rots = lidar_aug_matrix[..., :3, :3]
        extra_trans = lidar_aug_matrix[..., :3, 3]

        pts = _frustum()[None, None] - post_trans[:, :, None, None, None, :]
        pts = jnp.einsum('bnji,bndhwj->bndhwi', post_rots, pts)
        pts = jnp.concatenate([pts[..., :2] * pts[..., 2:3], pts[..., 2:3]], axis=-1)
        combine = jnp.swapaxes(
            jnp.linalg.solve(jnp.swapaxes(intrins, -1, -2), jnp.swapaxes(rots, -1, -2)),
            -1, -2)
        pts = jnp.einsum('bnij,bndhwj->bndhwi', combine, pts)
        pts = pts + trans[:, :, None, None, None, :]
        pts = jnp.einsum('bij,bndhwj->bndhwi', extra_rots, pts)
        pts = pts + extra_trans[:, None, None, None, None, :]

        dx = jnp.array([XB[2], YB[2], ZB[2]], dtype=jnp.float32)
        bx = jnp.array([XB[0] + XB[2] / 2.0, YB[0] + YB[2] / 2.0,
                        ZB[0] + ZB[2] / 2.0], dtype=jnp.float32)
        coords = ((pts - (bx - dx / 2.0)) / dx).astype(jnp.int32)
        coords = np.asarray(jax.device_get(coords))

    cx = coords[..., 0].ravel()
    cy = coords[..., 1].ravel()
    cz = coords[..., 2].ravel()
    kept = (cx >= 0) & (cx < NX) & (cy >= 0) & (cy < NY) & (cz >= 0) & (cz < NZ)
    return kept, cx, cy


def _plan(kept, cx, cy):
    """Sort kept points into (cx, cy-block) buckets, chunk, and balance across
    cores.  Returns per-core packed point-index grids, the shared chunk-length
    schedule, and per-(core, slot) bucket ids."""
    kidx = np.nonzero(kept)[0].astype(np.int64)
    kcx = cx[kidx].astype(np.int64)
    kcy = cy[kidx].astype(np.int64)
    bucket = kcx * NBLK + (kcy // W)
    order = np.argsort(bucket, kind="stable")
    sidx = kidx[order]                    # point indices sorted by bucket
    sbucket = bucket[order]
    cnt = np.bincount(sbucket, minlength=NX * NBLK)
    starts = np.concatenate([[0], np.cumsum(cnt)])

    # chunks: (bucket, ntiles, point_start, npts)
    chunks = []
    for b in np.nonzero(cnt)[0]:
        off = starts[b]
        rem = int(cnt[b])
        while rem > 0:
            npts = min(rem, SMAX * P)
            chunks.append((int(b), (npts + P - 1) // P, int(off), npts))
            off += npts
            rem -= npts
    chunks.sort(key=lambda c: -c[1])

    loads = [0] * NCORES
    percore = [[] for _ in range(NCORES)]
    for ch in chunks:
        k = loads.index(min(loads))
        percore[k].append(ch)
        loads[k] += ch[1]

    nslot = max(len(p) for p in percore)
    sched = [max(p[i][1] if i < len(p) else 0 for p in percore)
             for i in range(nslot)]
    ntile = sum(sched)
    nbatch = (ntile + BT - 1) // BT
    ntile_pad = nbatch * BT

    # per-core point-index grid (ntile_pad, P): -1 = padding
    idx_grids = []
    slot_buckets = []
    for k in range(NCORES):
        grid = np.full((ntile_pad, P), -1, dtype=np.int64)
        sb = np.full(nslot, -1, dtype=np.int64)
        t0 = 0
        for i, slen in enumerate(sched):
            if i < len(percore[k]):
                b, nt, off, npts = percore[k][i]
                sb[i] = b
                flat = grid[t0:t0 + slen].reshape(-1)
                flat[:npts] = sidx[off:off + npts]
            t0 += slen
        idx_grids.append(grid)
        slot_buckets.append(sb)
    return idx_grids, slot_buckets, sched, nbatch, nslot, ntile_pad


def _build(sched, nbatch, nslot, reps=1, mode="full"):
    """Build the SPMD Bacc program for the shared schedule.  reps>1 wraps the
    body in a device-side loop with a DATA-driven trip count (timing only);
    mode strips stages (timing only)."""
    ntile_pad = nbatch * BT
    ntile_used = sum(sched)
    nc = bacc.Bacc("TRN2", target_bir_lowering=False, num_devices=NCORES)
    xpk = nc.dram_tensor("xpk", [nbatch, P, BT * C], mybir.dt.bfloat16,
                         kind="ExternalInput")
    cypk = nc.dram_tensor("cypk", [P, ntile_pad + W], mybir.dt.bfloat16,
                          kind="ExternalInput")
    ngroup = (nslot + PSG - 1) // PSG
    out = nc.dram_tensor("out", [ngroup, W, PSG * C], mybir.dt.float32,
                         kind="ExternalOutput")

    with tile.TileContext(nc) as tc:
        from contextlib import ExitStack, nullcontext
        with (
            tc.tile_pool(name="const", bufs=1) as const_pool,
            tc.tile_pool(name="x", bufs=3) as x_pool,
            tc.tile_pool(name="oh", bufs=3) as oh_pool,
            tc.tile_pool(name="cp", bufs=4) as cp_pool,
            tc.tile_pool(name="ps", bufs=8, space="PSUM") as ps_pool,
        ):
            const_t = const_pool.tile([P, ntile_pad + W], mybir.dt.bfloat16)
            nc.sync.dma_start(out=const_t[:], in_=cypk[:, :])
            cy_t = const_t[:, :ntile_pad]
            iota_t = const_t[:, ntile_pad:]

            loop_cm = tc.For_i(0, reps, 1) if reps > 1 else nullcontext()
            with loop_cm:
                x_tiles = [None] * ntile_pad
                oh_tiles = [None] * ntile_pad
                staged = [False] * nbatch

                def stage_batch(bi):
                    t0 = bi * BT
                    used = min(BT, ntile_used - t0)
                    x_b = x_pool.tile([P, BT * C], mybir.dt.bfloat16)
                    nc.sync.dma_start(out=x_b[:, :used * C],
                                      in_=xpk[bi, :, :used * C])
                    oh_b = oh_pool.tile([P, BT * W], mybir.dt.bfloat16)
                    eng = nc.vector
                    if mode != "dma":
                        eng.tensor_tensor(
                            out=oh_b[:, :used * W].rearrange("p (t w) -> p t w", w=W),
                            in0=cy_t[:, t0:t0 + used][:, :, None].to_broadcast([P, used, W]),
                            in1=iota_t[:, None, :].to_broadcast([P, used, W]),
                            op=mybir.AluOpType.is_equal,
                        )
                    for j in range(used):
                        x_tiles[t0 + j] = x_b[:, j * C:(j + 1) * C]
                        oh_tiles[t0 + j] = oh_b[:, j * W:(j + 1) * W]
                    staged[bi] = True

                t = 0
                psum_t = None
                for si, slen in enumerate(sched):
                    gi, li = divmod(si, PSG)
                    if li == 0:
                        psum_t = ps_pool.tile([W, PSG * C], mybir.dt.float32,
                                              space="PSUM")
                    for j in range(slen):
                        bi = 0 if mode.startswith("pe") else t // BT
                        if not staged[bi]:
                            stage_batch(bi)
                        if mode not in ("dma", "dma_tt"):
                            tt = t % BT if mode.startswith("pe") else t
                            nc.tensor.matmul(
                                out=psum_t[:, li * C:(li + 1) * C],
                                lhsT=oh_tiles[tt], rhs=x_tiles[tt],
                                start=(j == 0), stop=(j == slen - 1),
                            )
                        t += 1
                    last_in_group = (li == PSG - 1) or (si == len(sched) - 1)
                    if not last_in_group:
                        continue
                    width = (li + 1) * C
                    if mode in ("full", "pe_full") or (
                            mode in ("dma_tt_mm", "pe_pure") and si == len(sched) - 1):
                        cp_t = cp_pool.tile([W, PSG * C], mybir.dt.float32)
                        nc.scalar.copy(out=cp_t[:, :width], in_=psum_t[:, :width])
                        nc.sync.dma_start(out=out[gi, :, :width], in_=cp_t[:, :width])
                    elif mode in ("dma", "dma_tt") and si == len(sched) - 1:
                        cp_t = cp_pool.tile([W, PSG * C], mybir.dt.float32)
                        nc.scalar.copy(out=cp_t[:, :C], in_=x_tiles[0][:W, :])
                        nc.sync.dma_start(out=out[gi, :, :C], in_=cp_t[:, :C])
    nc.compile()
    return nc


def _prepare(x, camera2lidar, camera_intrinsics, img_aug_matrix, lidar_aug_matrix):
    x = np.asarray(x)
    kept, cx, cy = _geometry(camera2lidar, camera_intrinsics,
                             img_aug_matrix, lidar_aug_matrix)
    idx_grids, slot_buckets, sched, nbatch, nslot, ntile_pad = _plan(kept, cx, cy)

    key = (tuple(sched), nbatch, nslot)
    if key not in _cache:
        _cache.clear()
        _cache[key] = _build(sched, nbatch, nslot)
    nc = _cache[key]

    xb = x.reshape(-1, C).astype(ml_dtypes.bfloat16)
    cyb = np.where(kept, cy & (W - 1), 0).astype(ml_dtypes.bfloat16)
    iota_np = np.broadcast_to(np.arange(W, dtype=np.float32), (P, W)) \
        .astype(ml_dtypes.bfloat16)

    in_maps = []
    for k in range(NCORES):
        grid = idx_grids[k]                       # (ntile_pad, P)
        safe = np.maximum(grid, 0)
        xg = xb[safe.reshape(-1)].reshape(ntile_pad, P, C)
        xg[grid.reshape(ntile_pad, P) < 0] = 0
        xpk = np.ascontiguousarray(
            xg.reshape(nbatch, BT, P, C).transpose(0, 2, 1, 3)
              .reshape(nbatch, P, BT * C))
        cyg = cyb[safe]                           # (ntile_pad, P) bf16
        cyg[grid < 0] = 0
        cypk = np.concatenate([cyg.T, iota_np], axis=1)  # (P, ntile_pad + W)
        in_maps.append({
            "xpk": xpk,
            "cypk": np.ascontiguousarray(cypk),
        })
    return nc, in_maps, slot_buckets, nslot


def _assemble(results, slot_buckets, nslot):
    outg = np.zeros((NY, NX, C), dtype=np.float32)   # (y, x, c) for fast adds
    for k in range(NCORES):
        part = results[k]["out"]                     # (ngroup, W, PSG*C)
        for si in range(nslot):
            b = int(slot_buckets[k][si])
            if b < 0:
                continue
            gi, li = divmod(si, PSG)
            gx, blk = divmod(b, NBLK)
            y0 = blk * W
            wv = min(W, NY - y0)
            outg[y0:y0 + wv, gx, :] += part[gi, :wv, li * C:(li + 1) * C]
    out = outg.transpose(2, 1, 0)                    # (C, NX, NY)
    return np.ascontiguousarray(out[None]).reshape(B, C * NZ, NX, NY)


def kernel(x, camera2lidar, camera_intrinsics, img_aug_matrix, lidar_aug_matrix):
    nc, in_maps, slot_buckets, nslot = _prepare(
        x, camera2lidar, camera_intrinsics, img_aug_matrix, lidar_aug_matrix)
    res = run_bass_kernel_spmd(nc, in_maps, core_ids=list(range(NCORES)))
    return _assemble(res.results, slot_buckets, nslot)


def _make_runner(nc, in_maps):
    """Build a reusable jitted PJRT runner for a compiled Bacc program.
    Returns run(), where run() executes once and returns wall seconds."""
    import time
    from jax.sharding import Mesh, PartitionSpec, NamedSharding
    from jax.experimental.shard_map import shard_map
    from concourse import bass2jax, mybir as _mb

    bass2jax.install_neuronx_cc_hook()

    partition_name = nc.partition_id_tensor.name if nc.partition_id_tensor else None
    in_names, out_names, out_avals, zero_outs = [], [], [], []
    for alloc in nc.m.functions[0].allocations:
        if not isinstance(alloc, _mb.MemoryLocationSet):
            continue
        name = alloc.memorylocations[0].name
        if alloc.kind == "ExternalInput":
            if name != partition_name:
                in_names.append(name)
        elif alloc.kind == "ExternalOutput":
            shape = tuple(alloc.tensor_shape)
            dtype = _mb.dt.np(alloc.dtype)
            out_names.append(name)
            out_avals.append(jax.core.ShapedArray(shape, dtype))
            zero_outs.append(np.zeros(shape, dtype))
    n_params = len(in_names)
    n_outs = len(out_avals)
    all_in_names = list(in_names) + list(out_names)
    if partition_name is not None:
        all_in_names.append(partition_name)

    def _body(*args):
        operands = list(args)
        if partition_name is not None:
            operands.append(bass2jax.partition_id_tensor())
        outs = bass2jax._bass_exec_p.bind(
            *operands,
            out_avals=tuple(out_avals),
            in_names=tuple(all_in_names),
            out_names=tuple(out_names),
            lowering_input_output_aliases=(),
            sim_require_finite=True,
            sim_require_nnan=True,
            nc=nc,
        )
        return tuple(outs)

    devices = jax.devices()[:NCORES]
    mesh = Mesh(np.asarray(devices), ("core",))
    in_specs = (PartitionSpec("core"),) * (n_params + n_outs)
    out_specs = (PartitionSpec("core"),) * n_outs
    donate = tuple(range(n_params, n_params + n_outs))
    sharded = jax.jit(
        shard_map(_body, mesh=mesh, in_specs=in_specs, out_specs=out_specs,
                  check_rep=False),
        donate_argnums=donate, keep_unused=True)

    shard = NamedSharding(mesh, PartitionSpec("core"))
    concat_in = [
        jax.device_put(
            np.concatenate([np.asarray(in_maps[c][nm]) for c in range(NCORES)],
                           axis=0), shard)
        for nm in in_names
    ]
    concat_zero_np = [
        np.zeros((NCORES * z.shape[0], *z.shape[1:]), z.dtype) for z in zero_outs
    ]

    def run():
        zeros_dev = [jax.device_put(z, shard) for z in concat_zero_np]
        jax.block_until_ready(zeros_dev)
        t0 = time.perf_counter()
        outs = sharded(*concat_in, *zeros_dev)
        jax.block_until_ready(outs)
        return time.perf_counter() - t0

    return run


def bench(inputs, iters=6, r1=64, r2=512, mode="full"):
    """Estimate per-execution HW time.  The axon dispatch floor is bimodal
    (~35.5 or ~76.5 ms per executable), so use rep counts large enough that
    the floor ambiguity is negligible: slope between r1 and r2 if the two
    executables landed in the same mode, else midpoint-corrected r2 value."""
    _nc1, in_maps, _, _ = _prepare(**inputs)
    kept_key = next(iter(_cache))
    sched, nbatch, nslot = list(kept_key[0]), kept_key[1], kept_key[2]
    nc_a = _build(sched, nbatch, nslot, reps=r1, mode=mode)
    nc_b = _build(sched, nbatch, nslot, reps=r2, mode=mode)

    run_a = _make_runner(nc_a, in_maps)
    run_b = _make_runner(nc_b, in_maps)
    run_a(); run_b()
    tas = [run_a() for _ in range(iters)]
    tbs = [run_b() for _ in range(iters)]
    ta, tb = min(tas), min(tbs)
    per_slope = (tb - ta) / (r2 - r1)
    floor_mid = 0.0795
    per_mid = (tb - floor_mid) / r2
    per = per_slope if abs(per_slope - per_mid) < 15e-6 else per_mid
    print(f"bench: x{r1} {ta*1e3:.1f} ms, x{r2} {tb*1e3:.1f} ms -> "
          f"slope {per_slope*1e6:.1f} us, mid {per_mid*1e6:.1f} us, "
          f"using {per*1e6:.1f} us")
    return per * 1e9
